# revision 22
# baseline (speedup 1.0000x reference)
"""Trainium2 Bass kernel for nn_MultiHeadAttention_48395691492077.

Reference (B=4, S=2048, D=1024, single head, anti-causal triu mask):
    qkv = x @ wqkv; q,k,v = split(qkv)
    scores = triu(q @ k^T / sqrt(B));  masked softmax over keys t >= s
    x2  = softmax(scores) @ v @ w_lin + b_lin + x
    out = relu(x2 @ w_ff1 + b_ff1) @ w_ff2 + b_ff2 + x2

Sharding: 8 cores = 4 batches x 2 query-halves. Each core computes the
full-batch v projection and attention for its own 1024 queries. The
program is identical on all cores (SPMD); per-core differences (which
queries, which mask pattern) are carried in the input data. The host
permutes x's sequence columns per-core so the core's OWN query columns
always sit at positions 0:1024 — the zq projection is then
branch-independent; only the scores/AV block indexing is parity-branched.

Device algebra (transposed so no on-chip transposes are needed):
    zqT[a,s] = wzq^T.T @ xT[:, 0:1024]  with wzq = (Wq @ Wk^T)/2
               (host-precomputed, QUERY-side folding: 1024 own queries)
    scoresT[t,s] = sum_a xT[a,t] * zqT[a,s]        (fp32r = fp22 matmul)
    expT = exp(scoresT) * mask01                   (no max-subtraction)
    den[s] = 64*ones[128,128].T @ expT (PE), rbs = 1/den (DVE)
    64*numerator^T[d,s] = (64*v)[t,d].T @ expT;  attn^T = num^T * rbs
    x2T = w_lin.T @ attn^T + (xT + b_lin);  hT = relu(w_ff1.T @ x2T + b_ff1)
    outT = w_ff2.T @ hT + x2T               (+ b_ff2 added on host)
Precision: scores chain in fp32r (fp22); v projection in fp8 e4m3 with
DoubleRow (wv scaled by 64 on host, cancelled via the 64-valued ones in
the softmax denominator); lin/ff1/ff2 in bf16 (fp32 PSUM everywhere).
"""

import numpy as np
import ml_dtypes

B, S, D = 4, 2048, 1024
NCORES = 8
BF16 = ml_dtypes.bfloat16
F8E4 = ml_dtypes.float8_e4m3   # TRN FP8_EXP4: max normal +-240

NT = S // 128            # 16 t-chunks
ND = D // 128            # 8 chunks of 128 along any D-sized dim

# global query-column starts of (sb0, sb1) per parity
SB_GLOBAL = {0: (0, 1536), 1: (512, 1024)}
# t-chunks each (parity, s-block) actually needs (branch-specialized)
SB_SLOTS = {
    0: {0: list(range(0, NT)), 1: list(range(12, NT))},
    1: {0: list(range(4, NT)), 1: list(range(8, NT))},
}
# per-parity column permutation (by 128-chunk): own queries first
PERM_CHUNKS = {
    0: [0, 1, 2, 3, 12, 13, 14, 15] + list(range(4, 12)),
    1: list(range(4, 12)) + [0, 1, 2, 3] + list(range(12, 16)),
}
# POS[p][true_chunk] = position of that key chunk in the permuted layout
POS = {p: {tc: i for i, tc in enumerate(PERM_CHUNKS[p])} for p in (0, 1)}


_COMPILED = None
_LAST_IN_MAPS = None


def _mask_order(parity: int):
    return [(sb, tc) for sb in (0, 1) for tc in SB_SLOTS[parity][sb]]


def _build_masks(parity: int) -> np.ndarray:
    """[20, 128, 512] bf16 multiplicative masks, one per processed block."""
    order = _mask_order(parity)
    m = np.zeros((len(order), 128, 512), np.float32)
    ii = np.arange(128)[:, None]
    jj = np.arange(512)[None, :]
    for k, (sb, tc) in enumerate(order):
        s0 = SB_GLOBAL[parity][sb]
        m[k] = ((128 * tc + ii) >= (s0 + jj)).astype(np.float32)
    return m.astype(BF16)


def _build_program():
    from contextlib import ExitStack
    import concourse.bacc as bacc
    import concourse.mybir as mybir
    import concourse.tile as tile

    f32 = mybir.dt.float32
    f16 = mybir.dt.float16
    b16 = mybir.dt.bfloat16
    f8 = mybir.dt.float8e4
    AF = mybir.ActivationFunctionType
    DR = mybir.MatmulPerfMode.DoubleRow

    nc = bacc.Bacc("TRN2", target_bir_lowering=False, debug=False,
                   num_devices=NCORES)

    xT8_d = nc.dram_tensor("xT8", [D, S], f8, kind="ExternalInput")
    xTf_d = nc.dram_tensor("xTf", [D, S], f16, kind="ExternalInput")
    wv8_d = nc.dram_tensor("wv8", [D, D], f8, kind="ExternalInput")
    # wzq host layout: [ND, 128, D] = a-chunk major so one chunk streams in
    wzq_d = nc.dram_tensor("wzq", [ND, 128, D], f16, kind="ExternalInput")
    xq_d = nc.dram_tensor("xq", [D, 1024], f16, kind="ExternalInput")
    wlin_d = nc.dram_tensor("wlin", [D, D], b16, kind="ExternalInput")
    wff1_d = nc.dram_tensor("wff1", [D, D], f8, kind="ExternalInput")
    wff2_d = nc.dram_tensor("wff2", [D, D], b16, kind="ExternalInput")
    diag_d = nc.dram_tensor("diag", [128, 128], b16, kind="ExternalInput")
    par_d = nc.dram_tensor("par", [1, 1], mybir.dt.uint32, kind="ExternalInput")
    bf1_d = nc.dram_tensor("bf1", [ND, 128], f32, kind="ExternalInput")
    outT_d = nc.dram_tensor("outT", [D, 1024], f16, kind="ExternalOutput")

    with tile.TileContext(nc) as tc:
        es = ExitStack()
        with es:
            pp = es.enter_context(tc.tile_pool(name="persist", bufs=1))
            sp = es.enter_context(tc.tile_pool(name="stream", bufs=2))
            ps = es.enter_context(
                tc.tile_pool(name="ps", bufs=8, space="PSUM"))
            # attn survives into phase C
            pr1 = es.enter_context(tc.tile_pool(name="pR1", bufs=1))
            esB = es.enter_context(ExitStack())
            pb = esB.enter_context(tc.tile_pool(name="pB", bufs=1))
            esX = ExitStack()
            px = esX.enter_context(tc.tile_pool(name="pX", bufs=1,
                                                side="right"))
            esA = ExitStack()
            pa = esA.enter_context(tc.tile_pool(name="pA", bufs=1,
                                                side="right"))

            def psum():
                t = ps.tile([128, 512], f32, tag="mm", bufs=8, name="mmps")
                return t

            # ---- constants ----
            # ones carries the 64x v-scale compensation into the softmax
            # denominator: den = 64*sum(exp), rbs = 1/(64*sum(exp)).
            ones_sq = pp.tile([128, 128], b16, tag="ones_sq", bufs=1)
            nc.vector.memset(ones_sq[:], 64.0)
            # warm the PE HAM clock-gate while input DMAs are in flight
            wups = psum()
            for i in range(64):
                nc.tensor.matmul(wups[:, 0:128], ones_sq[:], ones_sq[:],
                                 start=(i == 0), stop=(i == 63))

            # ---- input loads ----
            def chunked(dram, c0=None, c1=None):
                ap = dram.ap() if c0 is None else dram.ap()[:, c0:c1]
                return ap.rearrange("(c p) n -> p c n", p=128)

            # v-projection inputs first, finest-first column chunks so
            # the first matmuls start early
            wv8_a = pa.tile([128, ND, D], f8, tag="wv8", bufs=1)
            xt8_a = pa.tile([128, ND, S], f8, tag="xt8", bufs=1)
            xtf_a = px.tile([128, ND, S], f16, tag="xtf", bufs=1)

            def xt8_load(c0, c1):
                nc.sync.dma_start(xt8_a[:, :, c0:c1], chunked(xT8_d, c0, c1))

            def xtf_load(c0, c1):
                nc.sync.dma_start(xtf_a[:, :, c0:c1], chunked(xTf_d, c0, c1))

            nc.sync.dma_start(wv8_a[:, :, 0:512], chunked(wv8_d, 0, 512))
            xt8_load(0, 128)
            nc.sync.dma_start(wv8_a[:, :, 512:1024], chunked(wv8_d, 512, 1024))
            for c0, c1 in ((128, 256), (256, 512), (512, 1024),
                           (1024, 1536), (1536, 2048)):
                xt8_load(c0, c1)
            xtf_load(0, 512)          # zq sb0 operand
            xtf_load(512, 1024)       # zq sb1 operand
            xtf_load(1024, 1536)
            xtf_load(1536, 2048)
            # b_ff1 laid out [128, ND]: bias column fc serves f-chunk fc
            bf1_t = pp.tile([128, ND], f32, tag="bf1", bufs=1)
            nc.sync.dma_start(bf1_t[:], bf1_d.ap().rearrange("c p -> p c"))
            diag_t = pp.tile([128, 128], b16, tag="diag", bufs=1)
            nc.sync.dma_start(diag_t[:], diag_d.ap())

            # ---- phase A: v (fp8 DoubleRow) over permuted chunks ----
            vt = [pb.tile([128, D], b16, tag=f"vt{t}", bufs=1, name=f"vt{t}")
                  for t in range(NT)]
            zq_t = px.tile([128, ND, 1024], f16, tag="zq", bufs=1)

            for t in range(NT):
                for vb in range(2):
                    vps = psum()
                    for dp in range(ND // 2):
                        nc.tensor.matmul(
                            vps[:],
                            xt8_a[:, 2 * dp:2 * dp + 2,
                                  t * 128:(t + 1) * 128],
                            wv8_a[:, 2 * dp:2 * dp + 2,
                                  vb * 512:(vb + 1) * 512],
                            start=(dp == 0), stop=(dp == ND // 2 - 1),
                            perf_mode=DR)
                    nc.vector.tensor_copy(
                        vt[t][:, vb * 512:(vb + 1) * 512], vps[:])

            # ---- zq projection (fp32r), streamed weight chunks ----
            for m in range(ND):
                wzq_s = pa.tile([128, D], f16, tag="wzqs", bufs=3,
                                name=f"wzqs{m}")
                nc.gpsimd.dma_start(wzq_s[:], wzq_d.ap()[m])
                for sb in range(2):
                    zps = psum()
                    for d in range(ND):
                        nc.tensor.matmul(
                            zps[:],
                            wzq_s[:, d * 128:(d + 1) * 128],
                            xtf_a[:, d, sb * 512:(sb + 1) * 512],
                            start=(d == 0), stop=(d == ND - 1))
                    nc.vector.tensor_copy(
                        zq_t[:, m, sb * 512:(sb + 1) * 512], zps[:])

            attn = [pr1.tile([128, 1024], b16, tag=f"at{d}", bufs=1,
                             name=f"at{d}") for d in range(ND)]

            # v/zq inputs no longer needed once phase B begins
            esA.close()

            def phase_b(parity):
                sb_slots = SB_SLOTS[parity]
                pos = POS[parity]
                # pass 1: scoresT -> exp -> diag mask, sb-outer; edge
                # blocks only compute the valid column range, tails are
                # zeroed so den/AV can stay full-width
                et = {}
                rbs = {}
                for sb in (0, 1):
                    q0c = SB_GLOBAL[parity][sb] // 128
                    slots = sb_slots[sb]
                    for tcn in slots:
                        kcnt = min(4, tcn - q0c + 1)
                        wd = kcnt * 128
                        pc0 = pos[tcn] * 128
                        scp = psum()
                        for a in range(ND):
                            nc.tensor.matmul(
                                scp[:, 0:wd],
                                xtf_a[:, a, pc0:pc0 + 128],
                                zq_t[:, a, sb * 512:sb * 512 + wd],
                                start=(a == 0), stop=(a == ND - 1))
                        e = pb.tile([128, 512], b16, tag=f"et{sb}_{tcn}",
                                    bufs=1, name=f"et{parity}_{sb}_{tcn}")
                        et[(sb, tcn)] = e
                        if wd < 512:
                            nc.vector.memset(e[:, wd:512], 0.0)
                        nc.scalar.activation(e[:, 0:wd], scp[:, 0:wd],
                                             AF.Exp)
                        if tcn - q0c <= 3:
                            nc.vector.tensor_mul(
                                e[:, wd - 128:wd], e[:, wd - 128:wd],
                                diag_t[:])
                    # den for this s-block immediately after its last tile
                    den_ps = psum()
                    for k, tcn in enumerate(slots):
                        nc.tensor.matmul(
                            den_ps[:], ones_sq[:], et[(sb, tcn)][:],
                            start=(k == 0), stop=(k == len(slots) - 1))
                    r = pb.tile([128, 512], f32, tag="rbs", bufs=2,
                                name=f"rbs{parity}_{sb}")
                    nc.vector.reciprocal(r[:], den_ps[:])
                    rbs[sb] = r

                for dc in range(ND):
                    avp = {sb: psum() for sb in (0, 1)}
                    for sb in (0, 1):
                        slots = sb_slots[sb]
                        for k, tcn in enumerate(slots):
                            nc.tensor.matmul(
                                avp[sb][:],
                                vt[pos[tcn]][:, dc * 128:(dc + 1) * 128],
                                et[(sb, tcn)][:],
                                start=(k == 0),
                                stop=(k == len(slots) - 1))
                    for sb in (0, 1):
                        nc.vector.tensor_mul(
                            attn[dc][:, sb * 512:(sb + 1) * 512],
                            avp[sb][:], rbs[sb][:])

            par_regs = nc.alloc_registers("par_regs")
            nc.regs_load(par_regs, par_d.ap()[0:1, 0:1])
            par = nc.snap(par_regs, donate=True, min_val=0, max_val=1)
            with tc.If(par < 1) as cmp:
                phase_b(0)
            with cmp.Else():
                phase_b(1)

            # ---- free xtf/zq; load phase-C weights into that space ----
            esX.close()
            pr = es.enter_context(tc.tile_pool(name="pAC", bufs=1,
                                               side="right"))
            wl_a = pr.tile([128, ND, D], b16, tag="wl", bufs=1)
            nc.sync.dma_start(wl_a[:], chunked(wlin_d))
            wf1_a = pr.tile([128, ND, D], f8, tag="wf1", bufs=1)
            nc.sync.dma_start(wf1_a[:], chunked(wff1_d))
            wf2_a = pr.tile([128, ND, D], b16, tag="wf2", bufs=1)
            nc.sync.dma_start(wf2_a[:], chunked(wff2_d))
            wlin_t = [wl_a[:, d] for d in range(ND)]
            wff2_t = [wf2_a[:, d] for d in range(ND)]

            # ---- free pB (vt/et); left pool for phase-C tiles ----
            esB.close()
            esC = es.enter_context(ExitStack())
            pc = esC.enter_context(tc.tile_pool(name="pC", bufs=1))

            x2f = [pc.tile([128, 1024], f32, tag=f"x2f{d}", bufs=1,
                           name=f"x2f{d}") for d in range(ND)]
            x2b = pc.tile([128, ND, 1024], f8, tag="x2b", bufs=1)
            ht = [pc.tile([128, 1024], b16, tag=f"ht{d}", bufs=1,
                          name=f"ht{d}") for d in range(ND)]

            for oc in range(ND):
                for s2 in range(2):
                    xqt = pc.tile([128, 512], f16, tag="xqt", bufs=4,
                                  name=f"xqt{oc}_{s2}")
                    nc.sync.dma_start(
                        xqt[:],
                        xq_d.ap()[oc * 128:(oc + 1) * 128,
                                  s2 * 512:(s2 + 1) * 512])
                    cps = psum()
                    for d in range(ND):
                        nc.tensor.matmul(
                            cps[:],
                            wlin_t[d][:, oc * 128:(oc + 1) * 128],
                            attn[d][:, s2 * 512:(s2 + 1) * 512],
                            start=(d == 0), stop=(d == ND - 1))
                    cc = slice(s2 * 512, (s2 + 1) * 512)
                    nc.vector.tensor_add(x2f[oc][:, cc], cps[:], xqt[:])
                    nc.vector.tensor_copy(x2b[:, oc, cc], x2f[oc][:, cc])

            for fc in range(ND):
                for s2 in range(2):
                    cps = psum()
                    for dp in range(ND // 2):
                        nc.tensor.matmul(
                            cps[:],
                            wf1_a[:, 2 * dp:2 * dp + 2,
                                  fc * 128:(fc + 1) * 128],
                            x2b[:, 2 * dp:2 * dp + 2,
                                s2 * 512:(s2 + 1) * 512],
                            start=(dp == 0), stop=(dp == ND // 2 - 1),
                            perf_mode=DR)
                    cc = slice(s2 * 512, (s2 + 1) * 512)
                    nc.scalar.activation(ht[fc][:, cc], cps[:], AF.Relu,
                                         bias=bf1_t[:, fc:fc + 1],
                                         scale=1.0 / 64.0)

            for oc in range(ND):
                for s2 in range(2):
                    last = (oc == ND - 1 and s2 == 1)
                    cps = psum()
                    if not last:
                        for f in range(ND):
                            nc.tensor.matmul(
                                cps[:],
                                wff2_t[f][:, oc * 128:(oc + 1) * 128],
                                ht[f][:, s2 * 512:(s2 + 1) * 512],
                                start=(f == 0), stop=(f == ND - 1))
                        cc = slice(s2 * 512, (s2 + 1) * 512)
                        ot = pc.tile([128, 512], f16, tag="ot", bufs=4,
                                     name=f"ot{oc}_{s2}")
                        nc.vector.tensor_add(ot[:], cps[:], x2f[oc][:, cc])
                        nc.sync.dma_start(
                            outT_d.ap()[oc * 128:(oc + 1) * 128, cc], ot[:])
                    else:
                        # strip-mine the last tile so the final add+DMA
                        # drains in 128-col pieces
                        for k in range(4):
                            ks = slice(k * 128, (k + 1) * 128)
                            for f in range(ND):
                                nc.tensor.matmul(
                                    cps[:, ks],
                                    wff2_t[f][:, oc * 128:(oc + 1) * 128],
                                    ht[f][:, s2 * 512 + k * 128:
                                          s2 * 512 + (k + 1) * 128],
                                    start=(f == 0), stop=(f == ND - 1))
                        for k in range(4):
                            cg = slice(s2 * 512 + k * 128,
                                       s2 * 512 + (k + 1) * 128)
                            ot = pc.tile([128, 128], f16, tag="otl", bufs=4,
                                         name=f"otl{k}")
                            nc.vector.tensor_add(ot[:], cps[:, k * 128:
                                                 (k + 1) * 128],
                                                 x2f[oc][:, cg])
                            nc.sync.dma_start(
                                outT_d.ap()[oc * 128:(oc + 1) * 128, cg],
                                ot[:])

    nc.compile()
    return nc


def _get_program():
    global _COMPILED
    if _COMPILED is None:
        _COMPILED = _build_program()
    return _COMPILED


def kernel(x, wqkv, w_lin, b_lin, w_ff1, b_ff1, w_ff2, b_ff2):
    from concourse.bass_utils import run_bass_kernel_spmd

    x = np.asarray(x, np.float32)
    wqkv = np.asarray(wqkv, np.float32)
    Wq = wqkv[:, :D].astype(np.float64)
    Wk = wqkv[:, D:2 * D].astype(np.float64)
    Wv = wqkv[:, 2 * D:].astype(np.float32)

    # wzq [ND(a-chunk), 128(d rows of chunk? no: partition), D]:
    # lhsT for zq-proj needs [d(part), a(cols)] per a-chunk m: layout
    # wzq_h[m, p, d_col] = wzq[d_col? ...] -- build below explicitly.
    wzq = ((Wq @ Wk.T) / 2.0).astype(np.float32)      # [d, a]
    # chunk m holds columns a in [m*128,(m+1)*128): shape [128?]
    # SBUF tile is [128 part(d rows come in 8 d-chunks), D cols]:
    # tile[p, d*128 + j]?? -- we DMA wzq_d[m] (shape [128, D]) straight
    # into a [128, D] tile, so tile[p, c] = wzq_h[m, p, c]. The matmul
    # slices tile[:, d*128:(d+1)*128] as lhsT [128 part = d-rows of
    # chunk d, 128 cols = a-cols of chunk m]: so
    # wzq_h[m, p, d*128 + j] = wzq[d*128 + p, m*128 + j].
    wzq_h = np.empty((ND, 128, D), np.float32)
    for m in range(ND):
        for d in range(ND):
            wzq_h[m, :, d * 128:(d + 1) * 128] = \
                wzq[d * 128:(d + 1) * 128, m * 128:(m + 1) * 128]
    wv8 = np.clip(Wv * 64.0, -240, 240).astype(F8E4)
    wlin = np.asarray(w_lin, np.float32).astype(BF16)
    wff1 = np.clip(np.asarray(w_ff1, np.float32) * 64.0, -240, 240).astype(F8E4)
    wff2 = np.asarray(w_ff2, np.float32).astype(BF16)
    diag = (np.arange(128)[:, None] >= np.arange(128)[None, :]) \
        .astype(np.float32).astype(BF16)

    in_maps = []
    qcols_by_parity = {
        0: np.r_[0:512, 1536:2048],
        1: np.r_[512:1536],
    }
    perm_cols = {p: np.concatenate([np.arange(tc * 128, (tc + 1) * 128)
                                    for tc in PERM_CHUNKS[p]])
                 for p in (0, 1)}
    b_lin = np.asarray(b_lin, np.float32)
    b_ff1 = np.asarray(b_ff1, np.float32)
    b_ff2 = np.asarray(b_ff2, np.float32)
    bf1 = np.ascontiguousarray(b_ff1.reshape(ND, 128))
    for c in range(NCORES):
        b, h = c // 2, c % 2
        xT32 = np.ascontiguousarray(x[b].T)               # [D, S] f32
        xTp = np.ascontiguousarray(xT32[:, perm_cols[h]])  # permuted cols
        qcols = qcols_by_parity[h]
        qxT32 = np.ascontiguousarray(xT32[:, qcols])      # [D, 1024]
        in_maps.append({
            "xT8": np.clip(xTp, -240, 240).astype(F8E4),
            "xTf": xTp.astype(np.float16),
            "wv8": wv8,
            "wzq": wzq_h.astype(np.float16),
            "xq": (qxT32 + b_lin[:, None]).astype(np.float16),
            "wlin": wlin,
            "wff1": wff1,
            "wff2": wff2,
            "diag": diag,
            "bf1": bf1,
            "par": np.full((1, 1), h, np.uint32),
        })

    global _LAST_IN_MAPS
    _LAST_IN_MAPS = in_maps
    nc = _get_program()
    res = run_bass_kernel_spmd(nc, in_maps, core_ids=list(range(NCORES)))

    out = np.empty((B, S, D), np.float32)
    for c in range(NCORES):
        b, h = c // 2, c % 2
        ol = res.results[c]["outT"].astype(np.float32).T  # [1024 s, D]
        if h == 0:
            out[b, 0:512] = ol[:512]
            out[b, 1536:2048] = ol[512:]
        else:
            out[b, 512:1536] = ol
    out += b_ff2[None, None, :]
    return out


# revision 23
# speedup vs baseline: 1.1663x; 1.1663x over previous
"""Trainium2 Bass kernel for nn_MultiHeadAttention_48395691492077.

Reference (B=4, S=2048, D=1024, single head, anti-causal triu mask):
    qkv = x @ wqkv; q,k,v = split(qkv)
    scores = triu(q @ k^T / sqrt(B));  masked softmax over keys t >= s
    x2  = softmax(scores) @ v @ w_lin + b_lin + x
    out = relu(x2 @ w_ff1 + b_ff1) @ w_ff2 + b_ff2 + x2

Sharding: 8 cores = 4 batches x 2 query-halves. Each core computes the
full-batch v projection and attention for its own 1024 queries. The
program is identical on all cores (SPMD); per-core differences (which
queries, which mask pattern) are carried in the input data. The host
permutes x's sequence columns per-core so the core's OWN query columns
always sit at positions 0:1024 — the zq projection is then
branch-independent; only the scores/AV block indexing is parity-branched.

Device algebra (transposed so no on-chip transposes are needed):
    zqT[a,s] = wzq^T.T @ xT[:, 0:1024]  with wzq = (Wq @ Wk^T)/2
               (host-precomputed, QUERY-side folding: 1024 own queries)
    scoresT[t,s] = sum_a xT[a,t] * zqT[a,s]        (fp32r = fp22 matmul)
    expT = exp(scoresT) * mask01                   (no max-subtraction)
    den[s] = 64*ones[128,128].T @ expT (PE), rbs = 1/den (DVE)
    64*numerator^T[d,s] = (64*v)[t,d].T @ expT;  attn^T = num^T * rbs
    x2T = w_lin.T @ attn^T + (xT + b_lin);  hT = relu(w_ff1.T @ x2T + b_ff1)
    outT = w_ff2.T @ hT + x2T               (+ b_ff2 added on host)
Precision: scores chain in fp32r (fp22); v projection in fp8 e4m3 with
DoubleRow (wv scaled by 64 on host, cancelled via the 64-valued ones in
the softmax denominator); lin/ff1/ff2 in bf16 (fp32 PSUM everywhere).
"""

import numpy as np
import ml_dtypes

B, S, D = 4, 2048, 1024
NCORES = 8
BF16 = ml_dtypes.bfloat16
F8E4 = ml_dtypes.float8_e4m3   # TRN FP8_EXP4: max normal +-240

NT = S // 128            # 16 t-chunks
ND = D // 128            # 8 chunks of 128 along any D-sized dim

# global query-column starts of (sb0, sb1) per parity
SB_GLOBAL = {0: (0, 1536), 1: (512, 1024)}
# t-chunks each (parity, s-block) actually needs (branch-specialized)
SB_SLOTS = {
    0: {0: list(range(0, NT)), 1: list(range(12, NT))},
    1: {0: list(range(4, NT)), 1: list(range(8, NT))},
}
# per-parity column permutation (by 128-chunk): own queries first
PERM_CHUNKS = {
    0: [0, 1, 2, 3, 12, 13, 14, 15] + list(range(4, 12)),
    1: list(range(4, 12)) + [0, 1, 2, 3] + list(range(12, 16)),
}
# POS[p][true_chunk] = position of that key chunk in the permuted layout
POS = {p: {tc: i for i, tc in enumerate(PERM_CHUNKS[p])} for p in (0, 1)}


_COMPILED = None
_LAST_IN_MAPS = None


def _mask_order(parity: int):
    return [(sb, tc) for sb in (0, 1) for tc in SB_SLOTS[parity][sb]]


def _build_masks(parity: int) -> np.ndarray:
    """[20, 128, 512] bf16 multiplicative masks, one per processed block."""
    order = _mask_order(parity)
    m = np.zeros((len(order), 128, 512), np.float32)
    ii = np.arange(128)[:, None]
    jj = np.arange(512)[None, :]
    for k, (sb, tc) in enumerate(order):
        s0 = SB_GLOBAL[parity][sb]
        m[k] = ((128 * tc + ii) >= (s0 + jj)).astype(np.float32)
    return m.astype(BF16)


def _build_program():
    from contextlib import ExitStack
    import concourse.bacc as bacc
    import concourse.mybir as mybir
    import concourse.tile as tile

    f32 = mybir.dt.float32
    f16 = mybir.dt.float16
    b16 = mybir.dt.bfloat16
    f8 = mybir.dt.float8e4
    AF = mybir.ActivationFunctionType
    DR = mybir.MatmulPerfMode.DoubleRow

    nc = bacc.Bacc("TRN2", target_bir_lowering=False, debug=False,
                   num_devices=NCORES)

    xT8_d = nc.dram_tensor("xT8", [D, S], f8, kind="ExternalInput")
    xTf_d = nc.dram_tensor("xTf", [D, S], f16, kind="ExternalInput")
    wv8_d = nc.dram_tensor("wv8", [D, D], f8, kind="ExternalInput")
    # wzq host layout: [ND, 128, D] = a-chunk major so one chunk streams in
    wzq_d = nc.dram_tensor("wzq", [ND, 128, D], f16, kind="ExternalInput")
    xq_d = nc.dram_tensor("xq", [D, 1024], f16, kind="ExternalInput")
    wlin_d = nc.dram_tensor("wlin", [D, D], b16, kind="ExternalInput")
    wff1_d = nc.dram_tensor("wff1", [D, D], f8, kind="ExternalInput")
    wff2_d = nc.dram_tensor("wff2", [D, D], b16, kind="ExternalInput")
    diag_d = nc.dram_tensor("diag", [128, 128], b16, kind="ExternalInput")
    par_d = nc.dram_tensor("par", [1, 1], mybir.dt.uint32, kind="ExternalInput")
    bf1_d = nc.dram_tensor("bf1", [ND, 128], f32, kind="ExternalInput")
    outT_d = nc.dram_tensor("outT", [D, 1024], f16, kind="ExternalOutput")

    with tile.TileContext(nc) as tc:
        es = ExitStack()
        with es:
            pp = es.enter_context(tc.tile_pool(name="persist", bufs=1))
            sp = es.enter_context(tc.tile_pool(name="stream", bufs=2))
            ps = es.enter_context(
                tc.tile_pool(name="ps", bufs=8, space="PSUM"))
            # attn survives into phase C
            pr1 = es.enter_context(tc.tile_pool(name="pR1", bufs=1))
            esB = es.enter_context(ExitStack())
            pb = esB.enter_context(tc.tile_pool(name="pB", bufs=1))
            esX = ExitStack()
            px = esX.enter_context(tc.tile_pool(name="pX", bufs=1,
                                                side="right"))
            esA = ExitStack()
            pa = esA.enter_context(tc.tile_pool(name="pA", bufs=1,
                                                side="right"))

            def psum():
                t = ps.tile([128, 512], f32, tag="mm", bufs=8, name="mmps")
                return t

            # ---- constants ----
            # ones carries the 64x v-scale compensation into the softmax
            # denominator: den = 64*sum(exp), rbs = 1/(64*sum(exp)).
            ones_sq = pp.tile([128, 128], b16, tag="ones_sq", bufs=1)
            nc.vector.memset(ones_sq[:], 64.0)
            # warm the PE HAM clock-gate while input DMAs are in flight
            wups = psum()
            for i in range(64):
                nc.tensor.matmul(wups[:, 0:128], ones_sq[:], ones_sq[:],
                                 start=(i == 0), stop=(i == 63))

            # ---- input loads ----
            def chunked(dram, c0=None, c1=None):
                ap = dram.ap() if c0 is None else dram.ap()[:, c0:c1]
                return ap.rearrange("(c p) n -> p c n", p=128)

            # v-projection inputs first, finest-first column chunks so
            # the first matmuls start early
            wv8_a = pa.tile([128, ND, D], f8, tag="wv8", bufs=1)
            xt8_a = pa.tile([128, ND, S], f8, tag="xt8", bufs=1)
            xtf_a = px.tile([128, ND, S], f16, tag="xtf", bufs=1)

            def xt8_load(c0, c1):
                nc.sync.dma_start(xt8_a[:, :, c0:c1], chunked(xT8_d, c0, c1))

            def xtf_load(c0, c1):
                nc.sync.dma_start(xtf_a[:, :, c0:c1], chunked(xTf_d, c0, c1))

            nc.sync.dma_start(wv8_a[:, :, 0:512], chunked(wv8_d, 0, 512))
            xt8_load(0, 128)
            nc.sync.dma_start(wv8_a[:, :, 512:1024], chunked(wv8_d, 512, 1024))
            for c0, c1 in ((128, 256), (256, 512), (512, 1024)):
                xt8_load(c0, c1)
            xtf_load(0, 512)          # zq sb0 operand
            xt8_load(1024, 1536)
            xt8_load(1536, 2048)
            xtf_load(512, 1024)       # zq sb1 operand
            xtf_load(1024, 1536)
            xtf_load(1536, 2048)
            # b_ff1 laid out [128, ND]: bias column fc serves f-chunk fc
            bf1_t = pp.tile([128, ND], f32, tag="bf1", bufs=1)
            nc.sync.dma_start(bf1_t[:], bf1_d.ap().rearrange("c p -> p c"))
            diag_t = pp.tile([128, 128], b16, tag="diag", bufs=1)
            nc.sync.dma_start(diag_t[:], diag_d.ap())

            # ---- phase A: v (fp8 DoubleRow) over permuted chunks ----
            vt = [pb.tile([128, D], b16, tag=f"vt{t}", bufs=1, name=f"vt{t}")
                  for t in range(NT)]
            zq_t = px.tile([128, ND, 1024], f16, tag="zq", bufs=1)

            for t in range(NT):
                for vb in range(2):
                    vps = psum()
                    for dp in range(ND // 2):
                        nc.tensor.matmul(
                            vps[:],
                            xt8_a[:, 2 * dp:2 * dp + 2,
                                  t * 128:(t + 1) * 128],
                            wv8_a[:, 2 * dp:2 * dp + 2,
                                  vb * 512:(vb + 1) * 512],
                            start=(dp == 0), stop=(dp == ND // 2 - 1),
                            perf_mode=DR)
                    nc.vector.tensor_copy(
                        vt[t][:, vb * 512:(vb + 1) * 512], vps[:])

            # ---- zq projection (fp32r), streamed weight chunks ----
            for m in range(ND):
                wzq_s = pa.tile([128, D], f16, tag="wzqs", bufs=3,
                                name=f"wzqs{m}")
                nc.gpsimd.dma_start(wzq_s[:], wzq_d.ap()[m])
                for sb in range(2):
                    zps = psum()
                    for d in range(ND):
                        nc.tensor.matmul(
                            zps[:],
                            wzq_s[:, d * 128:(d + 1) * 128],
                            xtf_a[:, d, sb * 512:(sb + 1) * 512],
                            start=(d == 0), stop=(d == ND - 1))
                    nc.vector.tensor_copy(
                        zq_t[:, m, sb * 512:(sb + 1) * 512], zps[:])

            attn = [pr1.tile([128, 1024], b16, tag=f"at{d}", bufs=1,
                             name=f"at{d}") for d in range(ND)]

            # v/zq inputs no longer needed once phase B begins
            esA.close()

            def phase_b(parity):
                sb_slots = SB_SLOTS[parity]
                pos = POS[parity]
                # pass 1: scoresT -> exp -> diag mask, sb-outer; edge
                # blocks only compute the valid column range, tails are
                # zeroed so den/AV can stay full-width
                et = {}
                rbs = {}
                for sb in (0, 1):
                    q0c = SB_GLOBAL[parity][sb] // 128
                    slots = sb_slots[sb]
                    for tcn in slots:
                        kcnt = min(4, tcn - q0c + 1)
                        wd = kcnt * 128
                        pc0 = pos[tcn] * 128
                        scp = psum()
                        for a in range(ND):
                            nc.tensor.matmul(
                                scp[:, 0:wd],
                                xtf_a[:, a, pc0:pc0 + 128],
                                zq_t[:, a, sb * 512:sb * 512 + wd],
                                start=(a == 0), stop=(a == ND - 1))
                        e = pb.tile([128, 512], b16, tag=f"et{sb}_{tcn}",
                                    bufs=1, name=f"et{parity}_{sb}_{tcn}")
                        et[(sb, tcn)] = e
                        if wd < 512:
                            nc.vector.memset(e[:, wd:512], 0.0)
                        nc.scalar.activation(e[:, 0:wd], scp[:, 0:wd],
                                             AF.Exp)
                        if tcn - q0c <= 3:
                            nc.vector.tensor_mul(
                                e[:, wd - 128:wd], e[:, wd - 128:wd],
                                diag_t[:])
                    # den for this s-block immediately after its last tile
                    den_ps = psum()
                    for k, tcn in enumerate(slots):
                        nc.tensor.matmul(
                            den_ps[:], ones_sq[:], et[(sb, tcn)][:],
                            start=(k == 0), stop=(k == len(slots) - 1))
                    r = pb.tile([128, 512], f32, tag="rbs", bufs=2,
                                name=f"rbs{parity}_{sb}")
                    nc.vector.reciprocal(r[:], den_ps[:])
                    rbs[sb] = r

                for dc in range(ND):
                    avp = {sb: psum() for sb in (0, 1)}
                    for sb in (0, 1):
                        slots = sb_slots[sb]
                        for k, tcn in enumerate(slots):
                            nc.tensor.matmul(
                                avp[sb][:],
                                vt[pos[tcn]][:, dc * 128:(dc + 1) * 128],
                                et[(sb, tcn)][:],
                                start=(k == 0),
                                stop=(k == len(slots) - 1))
                    for sb in (0, 1):
                        nc.vector.tensor_mul(
                            attn[dc][:, sb * 512:(sb + 1) * 512],
                            avp[sb][:], rbs[sb][:])

            par_regs = nc.alloc_registers("par_regs")
            nc.regs_load(par_regs, par_d.ap()[0:1, 0:1])
            par = nc.snap(par_regs, donate=True, min_val=0, max_val=1)
            with tc.If(par < 1) as cmp:
                phase_b(0)
            with cmp.Else():
                phase_b(1)

            # ---- free xtf/zq; load phase-C weights into that space ----
            esX.close()
            pr = es.enter_context(tc.tile_pool(name="pAC", bufs=1,
                                               side="right"))
            wl_a = pr.tile([128, ND, D], b16, tag="wl", bufs=1)
            nc.sync.dma_start(wl_a[:], chunked(wlin_d))
            wf1_a = pr.tile([128, ND, D], f8, tag="wf1", bufs=1)
            nc.sync.dma_start(wf1_a[:], chunked(wff1_d))
            wf2_a = pr.tile([128, ND, D], b16, tag="wf2", bufs=1)
            nc.sync.dma_start(wf2_a[:], chunked(wff2_d))
            wlin_t = [wl_a[:, d] for d in range(ND)]
            wff2_t = [wf2_a[:, d] for d in range(ND)]

            # ---- free pB (vt/et); left pool for phase-C tiles ----
            esB.close()
            esC = es.enter_context(ExitStack())
            pc = esC.enter_context(tc.tile_pool(name="pC", bufs=1))

            x2f = [pc.tile([128, 1024], f32, tag=f"x2f{d}", bufs=1,
                           name=f"x2f{d}") for d in range(ND)]
            x2b = pc.tile([128, ND, 1024], f8, tag="x2b", bufs=1)
            ht = [pc.tile([128, 1024], b16, tag=f"ht{d}", bufs=1,
                          name=f"ht{d}") for d in range(ND)]

            for oc in range(ND):
                for s2 in range(2):
                    xqt = pc.tile([128, 512], f16, tag="xqt", bufs=4,
                                  name=f"xqt{oc}_{s2}")
                    nc.sync.dma_start(
                        xqt[:],
                        xq_d.ap()[oc * 128:(oc + 1) * 128,
                                  s2 * 512:(s2 + 1) * 512])
                    cps = psum()
                    for d in range(ND):
                        nc.tensor.matmul(
                            cps[:],
                            wlin_t[d][:, oc * 128:(oc + 1) * 128],
                            attn[d][:, s2 * 512:(s2 + 1) * 512],
                            start=(d == 0), stop=(d == ND - 1))
                    cc = slice(s2 * 512, (s2 + 1) * 512)
                    nc.vector.tensor_add(x2f[oc][:, cc], cps[:], xqt[:])
                    nc.vector.tensor_copy(x2b[:, oc, cc], x2f[oc][:, cc])

            for fc in range(ND):
                for s2 in range(2):
                    cps = psum()
                    for dp in range(ND // 2):
                        nc.tensor.matmul(
                            cps[:],
                            wf1_a[:, 2 * dp:2 * dp + 2,
                                  fc * 128:(fc + 1) * 128],
                            x2b[:, 2 * dp:2 * dp + 2,
                                s2 * 512:(s2 + 1) * 512],
                            start=(dp == 0), stop=(dp == ND // 2 - 1),
                            perf_mode=DR)
                    cc = slice(s2 * 512, (s2 + 1) * 512)
                    nc.scalar.activation(ht[fc][:, cc], cps[:], AF.Relu,
                                         bias=bf1_t[:, fc:fc + 1],
                                         scale=1.0 / 64.0)

            for oc in range(ND):
                for s2 in range(2):
                    cps = psum()
                    for f in range(ND):
                        nc.tensor.matmul(
                            cps[:],
                            wff2_t[f][:, oc * 128:(oc + 1) * 128],
                            ht[f][:, s2 * 512:(s2 + 1) * 512],
                            start=(f == 0), stop=(f == ND - 1))
                    cc = slice(s2 * 512, (s2 + 1) * 512)
                    ot = pc.tile([128, 512], f16, tag="ot", bufs=4,
                                 name=f"ot{oc}_{s2}")
                    nc.vector.tensor_add(ot[:], cps[:], x2f[oc][:, cc])
                    nc.sync.dma_start(
                        outT_d.ap()[oc * 128:(oc + 1) * 128, cc], ot[:])

    nc.compile()
    return nc


def _get_program():
    global _COMPILED
    if _COMPILED is None:
        _COMPILED = _build_program()
    return _COMPILED


def kernel(x, wqkv, w_lin, b_lin, w_ff1, b_ff1, w_ff2, b_ff2):
    from concourse.bass_utils import run_bass_kernel_spmd

    x = np.asarray(x, np.float32)
    wqkv = np.asarray(wqkv, np.float32)
    Wq = wqkv[:, :D].astype(np.float64)
    Wk = wqkv[:, D:2 * D].astype(np.float64)
    Wv = wqkv[:, 2 * D:].astype(np.float32)

    # wzq [ND(a-chunk), 128(d rows of chunk? no: partition), D]:
    # lhsT for zq-proj needs [d(part), a(cols)] per a-chunk m: layout
    # wzq_h[m, p, d_col] = wzq[d_col? ...] -- build below explicitly.
    wzq = ((Wq @ Wk.T) / 2.0).astype(np.float32)      # [d, a]
    # chunk m holds columns a in [m*128,(m+1)*128): shape [128?]
    # SBUF tile is [128 part(d rows come in 8 d-chunks), D cols]:
    # tile[p, d*128 + j]?? -- we DMA wzq_d[m] (shape [128, D]) straight
    # into a [128, D] tile, so tile[p, c] = wzq_h[m, p, c]. The matmul
    # slices tile[:, d*128:(d+1)*128] as lhsT [128 part = d-rows of
    # chunk d, 128 cols = a-cols of chunk m]: so
    # wzq_h[m, p, d*128 + j] = wzq[d*128 + p, m*128 + j].
    wzq_h = np.empty((ND, 128, D), np.float32)
    for m in range(ND):
        for d in range(ND):
            wzq_h[m, :, d * 128:(d + 1) * 128] = \
                wzq[d * 128:(d + 1) * 128, m * 128:(m + 1) * 128]
    wv8 = np.clip(Wv * 64.0, -240, 240).astype(F8E4)
    wlin = np.asarray(w_lin, np.float32).astype(BF16)
    wff1 = np.clip(np.asarray(w_ff1, np.float32) * 64.0, -240, 240).astype(F8E4)
    wff2 = np.asarray(w_ff2, np.float32).astype(BF16)
    diag = (np.arange(128)[:, None] >= np.arange(128)[None, :]) \
        .astype(np.float32).astype(BF16)

    in_maps = []
    qcols_by_parity = {
        0: np.r_[0:512, 1536:2048],
        1: np.r_[512:1536],
    }
    perm_cols = {p: np.concatenate([np.arange(tc * 128, (tc + 1) * 128)
                                    for tc in PERM_CHUNKS[p]])
                 for p in (0, 1)}
    b_lin = np.asarray(b_lin, np.float32)
    b_ff1 = np.asarray(b_ff1, np.float32)
    b_ff2 = np.asarray(b_ff2, np.float32)
    bf1 = np.ascontiguousarray(b_ff1.reshape(ND, 128))
    for c in range(NCORES):
        b, h = c // 2, c % 2
        xT32 = np.ascontiguousarray(x[b].T)               # [D, S] f32
        xTp = np.ascontiguousarray(xT32[:, perm_cols[h]])  # permuted cols
        qcols = qcols_by_parity[h]
        qxT32 = np.ascontiguousarray(xT32[:, qcols])      # [D, 1024]
        in_maps.append({
            "xT8": np.clip(xTp, -240, 240).astype(F8E4),
            "xTf": xTp.astype(np.float16),
            "wv8": wv8,
            "wzq": wzq_h.astype(np.float16),
            "xq": (qxT32 + b_lin[:, None]).astype(np.float16),
            "wlin": wlin,
            "wff1": wff1,
            "wff2": wff2,
            "diag": diag,
            "bf1": bf1,
            "par": np.full((1, 1), h, np.uint32),
        })

    global _LAST_IN_MAPS
    _LAST_IN_MAPS = in_maps
    nc = _get_program()
    res = run_bass_kernel_spmd(nc, in_maps, core_ids=list(range(NCORES)))

    out = np.empty((B, S, D), np.float32)
    for c in range(NCORES):
        b, h = c // 2, c % 2
        ol = res.results[c]["outT"].astype(np.float32).T  # [1024 s, D]
        if h == 0:
            out[b, 0:512] = ol[:512]
            out[b, 1536:2048] = ol[512:]
        else:
            out[b, 512:1536] = ol
    out += b_ff2[None, None, :]
    return out


# revision 25
# speedup vs baseline: 1.1862x; 1.0170x over previous
"""Trainium2 Bass kernel for nn_MultiHeadAttention_48395691492077.

Reference (B=4, S=2048, D=1024, single head, anti-causal triu mask):
    qkv = x @ wqkv; q,k,v = split(qkv)
    scores = triu(q @ k^T / sqrt(B));  masked softmax over keys t >= s
    x2  = softmax(scores) @ v @ w_lin + b_lin + x
    out = relu(x2 @ w_ff1 + b_ff1) @ w_ff2 + b_ff2 + x2

Sharding: 8 cores = 4 batches x 2 query-halves. Each core computes the
full-batch v projection and attention for its own 1024 queries. The
program is identical on all cores (SPMD); per-core differences (which
queries, which key-block geometry) are carried in the input data. The
host permutes x's sequence columns per-core so the core's OWN query
columns always sit at positions 0:1024 — the zq projection is then
branch-independent; only the scores/AV block indexing is parity-branched.

Device algebra (transposed so no on-chip transposes are needed):
    zqT[a,s] = wzq^T.T @ xT[:, 0:1024]  with wzq = (Wq @ Wk^T)/2
               (host-precomputed, QUERY-side folding: only 1024 queries,
               half the FLOPs of key-side folding)
    scoresT[t,s] = sum_a xT[a,t] * zqT[a,s]           (fp16 matmul)
    expT = exp(scoresT); triangle-edge blocks compute only their valid
           column range, zero the tail, and multiply the single shared
           [128,128] i>=j diagonal mask on the partial sub-block
    den[s] = ones(=64)[128,128].T @ expT (PE), rbs = 1/den (DVE)
    64*num^T[d,s] = (64*v)[t,d].T @ expT;  attn^T = num^T * rbs
    x2T = w_lin.T @ attn^T + (xT + b_lin);  hT = relu(w_ff1.T @ x2T/64
          + b_ff1);  outT = w_ff2.T @ hT + x2T   (+ b_ff2 on host)

Precision/engine plan (rel err ~0.0172 < 2e-2 gate):
  - scores chain (x, wzq, zq) in fp16 — cheap accuracy, halves DMA
  - v projection in fp8 e4m3 with DoubleRow perf mode (wv scaled by 64
    on host; the 64 cancels via the 64-valued ones in the softmax
    denominator); ff1 likewise fp8 DoubleRow with the 1/64 descale
    folded into the Relu activation's scale
  - lin/ff2 bf16 (fp32 PSUM everywhere); residual xq and output in fp16
"""

import numpy as np
import ml_dtypes

B, S, D = 4, 2048, 1024
NCORES = 8
BF16 = ml_dtypes.bfloat16
F8E4 = ml_dtypes.float8_e4m3   # TRN FP8_EXP4: max normal +-240

NT = S // 128            # 16 t-chunks
ND = D // 128            # 8 chunks of 128 along any D-sized dim

# global query-column starts of (sb0, sb1) per parity
SB_GLOBAL = {0: (0, 1536), 1: (512, 1024)}
# t-chunks each (parity, s-block) actually needs (branch-specialized)
SB_SLOTS = {
    0: {0: list(range(0, NT)), 1: list(range(12, NT))},
    1: {0: list(range(4, NT)), 1: list(range(8, NT))},
}
# per-parity column permutation (by 128-chunk): own queries first
PERM_CHUNKS = {
    0: [0, 1, 2, 3, 12, 13, 14, 15] + list(range(4, 12)),
    1: list(range(4, 12)) + [0, 1, 2, 3] + list(range(12, 16)),
}
# POS[p][true_chunk] = position of that key chunk in the permuted layout
POS = {p: {tc: i for i, tc in enumerate(PERM_CHUNKS[p])} for p in (0, 1)}


_COMPILED = None
_LAST_IN_MAPS = None


def _build_program():
    from contextlib import ExitStack
    import concourse.bacc as bacc
    import concourse.mybir as mybir
    import concourse.tile as tile

    f32 = mybir.dt.float32
    f16 = mybir.dt.float16
    b16 = mybir.dt.bfloat16
    f8 = mybir.dt.float8e4
    AF = mybir.ActivationFunctionType
    DR = mybir.MatmulPerfMode.DoubleRow

    nc = bacc.Bacc("TRN2", target_bir_lowering=False, debug=False,
                   num_devices=NCORES)

    xT8_d = nc.dram_tensor("xT8", [D, S], f8, kind="ExternalInput")
    xTf_d = nc.dram_tensor("xTf", [D, S], f16, kind="ExternalInput")
    wv8_d = nc.dram_tensor("wv8", [D, D], f8, kind="ExternalInput")
    # wzq host layout: [ND, 128, D] = a-chunk major so one chunk streams in
    wzq_d = nc.dram_tensor("wzq", [ND, 128, D], f16, kind="ExternalInput")
    xq_d = nc.dram_tensor("xq", [D, 1024], f16, kind="ExternalInput")
    wlin_d = nc.dram_tensor("wlin", [D, D], b16, kind="ExternalInput")
    wff1_d = nc.dram_tensor("wff1", [D, D], f8, kind="ExternalInput")
    wff2_d = nc.dram_tensor("wff2", [D, D], b16, kind="ExternalInput")
    diag_d = nc.dram_tensor("diag", [128, 128], b16, kind="ExternalInput")
    par_d = nc.dram_tensor("par", [1, 1], mybir.dt.uint32, kind="ExternalInput")
    bf1_d = nc.dram_tensor("bf1", [ND, 128], f32, kind="ExternalInput")
    outT_d = nc.dram_tensor("outT", [D, 1024], f16, kind="ExternalOutput")

    with tile.TileContext(nc) as tc:
        es = ExitStack()
        with es:
            pp = es.enter_context(tc.tile_pool(name="persist", bufs=1))
            sp = es.enter_context(tc.tile_pool(name="stream", bufs=2))
            ps = es.enter_context(
                tc.tile_pool(name="ps", bufs=8, space="PSUM"))
            # attn survives into phase C
            pr1 = es.enter_context(tc.tile_pool(name="pR1", bufs=1))
            esB = es.enter_context(ExitStack())
            pb = esB.enter_context(tc.tile_pool(name="pB", bufs=1))
            esX = ExitStack()
            px = esX.enter_context(tc.tile_pool(name="pX", bufs=1,
                                                side="right"))
            esA = ExitStack()
            pa = esA.enter_context(tc.tile_pool(name="pA", bufs=1,
                                                side="right"))

            def psum():
                t = ps.tile([128, 512], f32, tag="mm", bufs=8, name="mmps")
                return t

            # ---- constants ----
            # ones carries the 64x v-scale compensation into the softmax
            # denominator: den = 64*sum(exp), rbs = 1/(64*sum(exp)).
            ones_sq = pp.tile([128, 128], b16, tag="ones_sq", bufs=1)
            nc.vector.memset(ones_sq[:], 64.0)
            # warm the PE HAM clock-gate while input DMAs are in flight
            wups = psum()
            for i in range(64):
                nc.tensor.matmul(wups[:, 0:128], ones_sq[:], ones_sq[:],
                                 start=(i == 0), stop=(i == 63))

            # ---- input loads ----
            def chunked(dram, c0=None, c1=None):
                ap = dram.ap() if c0 is None else dram.ap()[:, c0:c1]
                return ap.rearrange("(c p) n -> p c n", p=128)

            # v-projection inputs first, finest-first column chunks so
            # the first matmuls start early
            wv8_a = pa.tile([128, ND, D], f8, tag="wv8", bufs=1)
            xt8_a = pa.tile([128, ND, S], f8, tag="xt8", bufs=1)
            xtf_a = px.tile([128, ND, S], f16, tag="xtf", bufs=1)

            def xt8_load(c0, c1):
                nc.sync.dma_start(xt8_a[:, :, c0:c1], chunked(xT8_d, c0, c1))

            def xtf_load(c0, c1):
                nc.sync.dma_start(xtf_a[:, :, c0:c1], chunked(xTf_d, c0, c1))

            nc.sync.dma_start(wv8_a[:, :, 0:512], chunked(wv8_d, 0, 512))
            xt8_load(0, 128)
            nc.sync.dma_start(wv8_a[:, :, 512:1024], chunked(wv8_d, 512, 1024))
            for c0, c1 in ((128, 256), (256, 512), (512, 1024)):
                xt8_load(c0, c1)
            xtf_load(0, 512)          # zq sb0 operand
            xt8_load(1024, 1536)
            xt8_load(1536, 2048)
            xtf_load(512, 1024)       # zq sb1 operand
            xtf_load(1024, 1536)
            xtf_load(1536, 2048)
            # b_ff1 laid out [128, ND]: bias column fc serves f-chunk fc
            bf1_t = pp.tile([128, ND], f32, tag="bf1", bufs=1)
            nc.sync.dma_start(bf1_t[:], bf1_d.ap().rearrange("c p -> p c"))
            diag_t = pp.tile([128, 128], b16, tag="diag", bufs=1)
            nc.sync.dma_start(diag_t[:], diag_d.ap())

            # ---- phase A: v (fp8 DoubleRow) over permuted chunks ----
            vt = [pb.tile([128, D], b16, tag=f"vt{t}", bufs=1, name=f"vt{t}")
                  for t in range(NT)]
            zq_t = px.tile([128, ND, 1024], f16, tag="zq", bufs=1)

            for t in range(NT):
                for vb in range(2):
                    vps = psum()
                    for dp in range(ND // 2):
                        nc.tensor.matmul(
                            vps[:],
                            xt8_a[:, 2 * dp:2 * dp + 2,
                                  t * 128:(t + 1) * 128],
                            wv8_a[:, 2 * dp:2 * dp + 2,
                                  vb * 512:(vb + 1) * 512],
                            start=(dp == 0), stop=(dp == ND // 2 - 1),
                            perf_mode=DR)
                    nc.vector.tensor_copy(
                        vt[t][:, vb * 512:(vb + 1) * 512], vps[:])

            # ---- zq projection (fp32r), streamed weight chunks ----
            for m in range(ND):
                wzq_s = pa.tile([128, D], f16, tag="wzqs", bufs=3,
                                name=f"wzqs{m}")
                nc.gpsimd.dma_start(wzq_s[:], wzq_d.ap()[m])
                for sb in range(2):
                    zps = psum()
                    for d in range(ND):
                        nc.tensor.matmul(
                            zps[:],
                            wzq_s[:, d * 128:(d + 1) * 128],
                            xtf_a[:, d, sb * 512:(sb + 1) * 512],
                            start=(d == 0), stop=(d == ND - 1))
                    nc.vector.tensor_copy(
                        zq_t[:, m, sb * 512:(sb + 1) * 512], zps[:])

            attn = [pr1.tile([128, 1024], b16, tag=f"at{d}", bufs=1,
                             name=f"at{d}") for d in range(ND)]

            # v/zq inputs no longer needed once phase B begins
            esA.close()

            def phase_b(parity):
                sb_slots = SB_SLOTS[parity]
                pos = POS[parity]
                # pass 1: scoresT -> exp -> diag mask, sb-outer; edge
                # blocks only compute the valid column range, tails are
                # zeroed so den/AV can stay full-width
                et = {}
                rbs = {}
                for sb in (0, 1):
                    q0c = SB_GLOBAL[parity][sb] // 128
                    slots = sb_slots[sb]
                    for tcn in slots:
                        kcnt = min(4, tcn - q0c + 1)
                        wd = kcnt * 128
                        pc0 = pos[tcn] * 128
                        scp = psum()
                        for a in range(ND):
                            nc.tensor.matmul(
                                scp[:, 0:wd],
                                xtf_a[:, a, pc0:pc0 + 128],
                                zq_t[:, a, sb * 512:sb * 512 + wd],
                                start=(a == 0), stop=(a == ND - 1))
                        e = pb.tile([128, 512], b16, tag=f"et{sb}_{tcn}",
                                    bufs=1, name=f"et{parity}_{sb}_{tcn}")
                        et[(sb, tcn)] = e
                        if wd < 512:
                            nc.vector.memset(e[:, wd:512], 0.0)
                        nc.scalar.activation(e[:, 0:wd], scp[:, 0:wd],
                                             AF.Exp)
                        if tcn - q0c <= 3:
                            nc.vector.tensor_mul(
                                e[:, wd - 128:wd], e[:, wd - 128:wd],
                                diag_t[:])
                    # den for this s-block immediately after its last tile
                    den_ps = psum()
                    for k, tcn in enumerate(slots):
                        nc.tensor.matmul(
                            den_ps[:], ones_sq[:], et[(sb, tcn)][:],
                            start=(k == 0), stop=(k == len(slots) - 1))
                    r = pb.tile([128, 512], f32, tag="rbs", bufs=2,
                                name=f"rbs{parity}_{sb}")
                    nc.vector.reciprocal(r[:], den_ps[:])
                    rbs[sb] = r

                for dc in range(ND):
                    avp = {sb: psum() for sb in (0, 1)}
                    for sb in (0, 1):
                        slots = sb_slots[sb]
                        for k, tcn in enumerate(slots):
                            nc.tensor.matmul(
                                avp[sb][:],
                                vt[pos[tcn]][:, dc * 128:(dc + 1) * 128],
                                et[(sb, tcn)][:],
                                start=(k == 0),
                                stop=(k == len(slots) - 1))
                    for sb in (0, 1):
                        nc.vector.tensor_mul(
                            attn[dc][:, sb * 512:(sb + 1) * 512],
                            avp[sb][:], rbs[sb][:])

            par_regs = nc.alloc_registers("par_regs")
            nc.regs_load(par_regs, par_d.ap()[0:1, 0:1])
            par = nc.snap(par_regs, donate=True, min_val=0, max_val=1)
            with tc.If(par < 1) as cmp:
                phase_b(0)
            with cmp.Else():
                phase_b(1)

            # ---- free xtf/zq; load phase-C weights into that space ----
            esX.close()
            pr = es.enter_context(tc.tile_pool(name="pAC", bufs=1,
                                               side="right"))
            wl_a = pr.tile([128, ND, D], b16, tag="wl", bufs=1)
            nc.sync.dma_start(wl_a[:], chunked(wlin_d))
            wf1_a = pr.tile([128, ND, D], f8, tag="wf1", bufs=1)
            nc.sync.dma_start(wf1_a[:], chunked(wff1_d))
            wf2_a = pr.tile([128, ND, D], b16, tag="wf2", bufs=1)
            nc.sync.dma_start(wf2_a[:], chunked(wff2_d))
            wlin_t = [wl_a[:, d] for d in range(ND)]
            wff2_t = [wf2_a[:, d] for d in range(ND)]

            # ---- free pB (vt/et); left pool for phase-C tiles ----
            esB.close()
            esC = es.enter_context(ExitStack())
            pc = esC.enter_context(tc.tile_pool(name="pC", bufs=1))

            x2f = [pc.tile([128, 1024], f32, tag=f"x2f{d}", bufs=1,
                           name=f"x2f{d}") for d in range(ND)]
            x2b = pc.tile([128, ND, 1024], f8, tag="x2b", bufs=1)
            ht = [pc.tile([128, 1024], b16, tag=f"ht{d}", bufs=1,
                          name=f"ht{d}") for d in range(ND)]

            for oc in range(ND):
                for s2 in range(2):
                    xqt = pc.tile([128, 512], f16, tag="xqt", bufs=4,
                                  name=f"xqt{oc}_{s2}")
                    nc.sync.dma_start(
                        xqt[:],
                        xq_d.ap()[oc * 128:(oc + 1) * 128,
                                  s2 * 512:(s2 + 1) * 512])
                    cps = psum()
                    for d in range(ND):
                        nc.tensor.matmul(
                            cps[:],
                            wlin_t[d][:, oc * 128:(oc + 1) * 128],
                            attn[d][:, s2 * 512:(s2 + 1) * 512],
                            start=(d == 0), stop=(d == ND - 1))
                    cc = slice(s2 * 512, (s2 + 1) * 512)
                    nc.vector.tensor_add(x2f[oc][:, cc], cps[:], xqt[:])
                    nc.vector.tensor_copy(x2b[:, oc, cc], x2f[oc][:, cc])

            for fc in range(ND):
                for s2 in range(2):
                    cps = psum()
                    for dp in range(ND // 2):
                        nc.tensor.matmul(
                            cps[:],
                            wf1_a[:, 2 * dp:2 * dp + 2,
                                  fc * 128:(fc + 1) * 128],
                            x2b[:, 2 * dp:2 * dp + 2,
                                s2 * 512:(s2 + 1) * 512],
                            start=(dp == 0), stop=(dp == ND // 2 - 1),
                            perf_mode=DR)
                    cc = slice(s2 * 512, (s2 + 1) * 512)
                    nc.scalar.activation(ht[fc][:, cc], cps[:], AF.Relu,
                                         bias=bf1_t[:, fc:fc + 1],
                                         scale=1.0 / 64.0)

            for oc in range(ND):
                for s2 in range(2):
                    cps = psum()
                    for f in range(ND):
                        nc.tensor.matmul(
                            cps[:],
                            wff2_t[f][:, oc * 128:(oc + 1) * 128],
                            ht[f][:, s2 * 512:(s2 + 1) * 512],
                            start=(f == 0), stop=(f == ND - 1))
                    cc = slice(s2 * 512, (s2 + 1) * 512)
                    ot = pc.tile([128, 512], f16, tag="ot", bufs=4,
                                 name=f"ot{oc}_{s2}")
                    nc.vector.tensor_add(ot[:], cps[:], x2f[oc][:, cc])
                    nc.sync.dma_start(
                        outT_d.ap()[oc * 128:(oc + 1) * 128, cc], ot[:])

    nc.compile()
    return nc


def _get_program():
    global _COMPILED
    if _COMPILED is None:
        _COMPILED = _build_program()
    return _COMPILED


def kernel(x, wqkv, w_lin, b_lin, w_ff1, b_ff1, w_ff2, b_ff2):
    from concourse.bass_utils import run_bass_kernel_spmd

    x = np.asarray(x, np.float32)
    wqkv = np.asarray(wqkv, np.float32)
    Wq = wqkv[:, :D].astype(np.float64)
    Wk = wqkv[:, D:2 * D].astype(np.float64)
    Wv = wqkv[:, 2 * D:].astype(np.float32)

    # wzq [ND(a-chunk), 128(d rows of chunk? no: partition), D]:
    # lhsT for zq-proj needs [d(part), a(cols)] per a-chunk m: layout
    # wzq_h[m, p, d_col] = wzq[d_col? ...] -- build below explicitly.
    wzq = ((Wq @ Wk.T) / 2.0).astype(np.float32)      # [d, a]
    # chunk m holds columns a in [m*128,(m+1)*128): shape [128?]
    # SBUF tile is [128 part(d rows come in 8 d-chunks), D cols]:
    # tile[p, d*128 + j]?? -- we DMA wzq_d[m] (shape [128, D]) straight
    # into a [128, D] tile, so tile[p, c] = wzq_h[m, p, c]. The matmul
    # slices tile[:, d*128:(d+1)*128] as lhsT [128 part = d-rows of
    # chunk d, 128 cols = a-cols of chunk m]: so
    # wzq_h[m, p, d*128 + j] = wzq[d*128 + p, m*128 + j].
    wzq_h = np.empty((ND, 128, D), np.float32)
    for m in range(ND):
        for d in range(ND):
            wzq_h[m, :, d * 128:(d + 1) * 128] = \
                wzq[d * 128:(d + 1) * 128, m * 128:(m + 1) * 128]
    wv8 = np.clip(Wv * 64.0, -240, 240).astype(F8E4)
    wlin = np.asarray(w_lin, np.float32).astype(BF16)
    wff1 = np.clip(np.asarray(w_ff1, np.float32) * 64.0, -240, 240).astype(F8E4)
    wff2 = np.asarray(w_ff2, np.float32).astype(BF16)
    diag = (np.arange(128)[:, None] >= np.arange(128)[None, :]) \
        .astype(np.float32).astype(BF16)

    in_maps = []
    qcols_by_parity = {
        0: np.r_[0:512, 1536:2048],
        1: np.r_[512:1536],
    }
    perm_cols = {p: np.concatenate([np.arange(tc * 128, (tc + 1) * 128)
                                    for tc in PERM_CHUNKS[p]])
                 for p in (0, 1)}
    b_lin = np.asarray(b_lin, np.float32)
    b_ff1 = np.asarray(b_ff1, np.float32)
    b_ff2 = np.asarray(b_ff2, np.float32)
    bf1 = np.ascontiguousarray(b_ff1.reshape(ND, 128))
    for c in range(NCORES):
        b, h = c // 2, c % 2
        xT32 = np.ascontiguousarray(x[b].T)               # [D, S] f32
        xTp = np.ascontiguousarray(xT32[:, perm_cols[h]])  # permuted cols
        qcols = qcols_by_parity[h]
        qxT32 = np.ascontiguousarray(xT32[:, qcols])      # [D, 1024]
        in_maps.append({
            "xT8": np.clip(xTp, -240, 240).astype(F8E4),
            "xTf": xTp.astype(np.float16),
            "wv8": wv8,
            "wzq": wzq_h.astype(np.float16),
            "xq": (qxT32 + b_lin[:, None]).astype(np.float16),
            "wlin": wlin,
            "wff1": wff1,
            "wff2": wff2,
            "diag": diag,
            "bf1": bf1,
            "par": np.full((1, 1), h, np.uint32),
        })

    global _LAST_IN_MAPS
    _LAST_IN_MAPS = in_maps
    nc = _get_program()
    res = run_bass_kernel_spmd(nc, in_maps, core_ids=list(range(NCORES)))

    out = np.empty((B, S, D), np.float32)
    for c in range(NCORES):
        b, h = c // 2, c % 2
        ol = res.results[c]["outT"].astype(np.float32).T  # [1024 s, D]
        if h == 0:
            out[b, 0:512] = ol[:512]
            out[b, 1536:2048] = ol[512:]
        else:
            out[b, 512:1536] = ol
    out += b_ff2[None, None, :]
    return out
